# revision 3
# baseline (speedup 1.0000x reference)
"""DescendantMax kernel for Trainium2 (8 NeuronCores, pure data parallel).

Tree structure (hardcoded from the problem spec): balanced 8-ary tree,
DEPTH=6 parent->child levels, BFS node numbering.  Level k starts at
s_k = (8^k - 1) / 7 and has 8^k nodes.  Children of the j-th node of
level k are the 8 consecutive nodes s_{k+1} + 8j ... 8j+8.  So each
level's "gather" is a contiguous reshape and the whole computation per
batch row is a chain of 8:1 contiguous-group max reductions, each
followed by an elementwise max with the parent level's input values.

Sharding: x is (64, 299593) f32; shard batch across 8 cores (8 rows
per core).  Each core:
  - levels 6..4 are processed per-row in partition-major layout
    [128, n/128] so the groups-of-8 stay within a partition's free dim,
  - level-3 results are repacked (SBUF->SBUF DMA) into a row-per-
    partition [8, 512] tile, and levels 3->2->1->0 are done for all 8
    rows in one shot,
  - the (unmodified) leaf level is passed through SBUF once: one HBM
    read + one HBM write per element, which is the memory roofline.
"""

import numpy as np

BRANCH = 8
DEPTH = 6
BATCH = 64
N_CORES = 8
ROWS = BATCH // N_CORES  # rows per core
# starts[k] = (8^k - 1) // 7 ; starts[DEPTH+1] == total node count
STARTS = [(BRANCH**k - 1) // (BRANCH - 1) for k in range(DEPTH + 2)]
N_NODES = STARTS[DEPTH + 1]  # 299593

_cache: dict = {}


def _build_nc():
    import concourse.bacc as bacc
    import concourse.mybir as mybir
    from concourse.tile import TileContext

    f32 = mybir.dt.float32
    AX = mybir.AxisListType.X

    # Bacc (not raw Bass): its compile() pipeline runs
    # generate_event_semaphores, which splits multi-wait sync_info into
    # EventSemaphore insts — TRN2 allows at most 1 wait per instruction.
    nc = bacc.Bacc(None, target_bir_lowering=False)
    x = nc.dram_tensor("x", [ROWS, N_NODES], f32, kind="ExternalInput")
    out = nc.dram_tensor("out", [ROWS, N_NODES], f32, kind="ExternalOutput")

    def dview(t, r, lvl):
        """DRAM AP for row r, level lvl, in partition-major [128, n/128]."""
        a, b = STARTS[lvl], STARTS[lvl + 1]
        return t[r, a:b].rearrange("(p f) -> p f", p=128)

    with TileContext(nc) as tc:
        with (
            tc.tile_pool(name="big", bufs=3) as big,
            tc.tile_pool(name="mid", bufs=3) as mid,
            tc.tile_pool(name="tail", bufs=1) as tailp,
        ):
            tail3 = tailp.tile([ROWS, 512], f32)
            for r in range(ROWS):
                # ---- level 6 (leaves): load, pass through, reduce -> m5
                t6 = big.tile([128, 2048], f32, tag="t6")
                nc.sync.dma_start(out=t6[:, :], in_=dview(x, r, 6))
                nc.sync.dma_start(out=dview(out, r, 6), in_=t6[:, :])
                m5 = mid.tile([128, 256], f32, tag="m5")
                nc.vector.reduce_max(
                    out=m5[:, :],
                    in_=t6[:, :].rearrange("p (g e) -> p g e", e=8),
                    axis=AX,
                )
                # ---- level 5
                x5 = mid.tile([128, 256], f32, tag="x5")
                nc.sync.dma_start(out=x5[:, :], in_=dview(x, r, 5))
                o5 = mid.tile([128, 256], f32, tag="o5")
                nc.vector.tensor_max(out=o5[:, :], in0=m5[:, :], in1=x5[:, :])
                nc.sync.dma_start(out=dview(out, r, 5), in_=o5[:, :])
                m4 = mid.tile([128, 32], f32, tag="m4")
                nc.vector.reduce_max(
                    out=m4[:, :],
                    in_=o5[:, :].rearrange("p (g e) -> p g e", e=8),
                    axis=AX,
                )
                # ---- level 4
                x4 = mid.tile([128, 32], f32, tag="x4")
                nc.sync.dma_start(out=x4[:, :], in_=dview(x, r, 4))
                o4 = mid.tile([128, 32], f32, tag="o4")
                nc.vector.tensor_max(out=o4[:, :], in0=m4[:, :], in1=x4[:, :])
                nc.sync.dma_start(out=dview(out, r, 4), in_=o4[:, :])
                m3 = mid.tile([128, 4], f32, tag="m3")
                nc.vector.reduce_max(
                    out=m3[:, :],
                    in_=o4[:, :].rearrange("p (g e) -> p g e", e=8),
                    axis=AX,
                )
                # ---- level 3
                x3 = mid.tile([128, 4], f32, tag="x3")
                nc.sync.dma_start(out=x3[:, :], in_=dview(x, r, 3))
                o3 = mid.tile([128, 4], f32, tag="o3")
                nc.vector.tensor_max(out=o3[:, :], in0=m3[:, :], in1=x3[:, :])
                nc.sync.dma_start(out=dview(out, r, 3), in_=o3[:, :])
                # repack this row's level-3 values to row-per-partition
                nc.sync.dma_start(out=tail3[r : r + 1, :], in_=o3[:, :])

            # ---- levels 3 -> 2 -> 1 -> 0, all rows at once ([ROWS, n])
            prev = tail3
            for lvl in (2, 1, 0):
                n = BRANCH**lvl
                m = tailp.tile([ROWS, n], f32, tag=f"m{lvl}t")
                nc.vector.reduce_max(
                    out=m[:, :],
                    in_=prev[:, :].rearrange("p (g e) -> p g e", e=8),
                    axis=AX,
                )
                xl = tailp.tile([ROWS, n], f32, tag=f"x{lvl}t")
                nc.sync.dma_start(
                    out=xl[:, :], in_=x[:, STARTS[lvl] : STARTS[lvl + 1]]
                )
                o = tailp.tile([ROWS, n], f32, tag=f"o{lvl}t")
                nc.vector.tensor_max(out=o[:, :], in0=m[:, :], in1=xl[:, :])
                nc.sync.dma_start(
                    out=out[:, STARTS[lvl] : STARTS[lvl + 1]], in_=o[:, :]
                )
                prev = o
    nc.compile()
    return nc


def _get_nc():
    if "nc" not in _cache:
        _cache["nc"] = _build_nc()
    return _cache["nc"]


def kernel(x, level_parents=None, level_children=None, **_ignored):
    from concourse.bass_utils import run_bass_kernel_spmd

    x = np.ascontiguousarray(np.asarray(x), dtype=np.float32)
    assert x.shape == (BATCH, N_NODES), x.shape

    nc = _get_nc()
    core_ids = list(range(N_CORES))
    in_maps = [
        {"x": x[i * ROWS : (i + 1) * ROWS]} for i in range(N_CORES)
    ]
    res = run_bass_kernel_spmd(nc, in_maps, core_ids)
    return np.concatenate([res.results[i]["out"] for i in range(N_CORES)], axis=0)
